# revision 11
# baseline (speedup 1.0000x reference)
"""TRN2 Bass kernel for nn_ODEModel (RK4 neural ODE, dense MLP vector field).

Strategy: 8-way DATA-parallel over the batch (32 rows/core), zero collectives.
All weights SBUF-resident (W2^T in fp8 = 8MB).  Per f-eval on each core:
  h1T = relu(W1^T chunks @ y)          32x [128,32] psum tiles, feature-major,
                                       cast to fp8 (e4m3)
  h2' = relu(h1 @ (256*W2^T) + 256*b2) batch-major [32, 2048] via Double-FP8
                                       (256-dim contraction per mm); bias as a
                                       17th DoubleRow mm; scale folded into W3
  h2T = PE-transpose(h2')              16x [32,128]->[128,32]
  pp  = (W3^T/256) chunks @ h2T        accumulated [4,32]; tanh(pp+b3) on ACT
  k   = tanh + poly(y)                 poly via 3 tiny matmuls, emitted late
RK4 combination in fp32 (master state); z-terms precomputed during the last
GEMM tile so only one DVE op separates tanh from the next eval's matmuls.
Output stored as [T-1, 4, 32] per core; host transposes/concats and
prepends y0.
"""
import sys

sys.path.insert(0, "/opt/trn_rl_repo")
import numpy as np
import ml_dtypes

import concourse.bass as bass
import concourse.bacc as bacc
import concourse.tile as tile
import concourse.mybir as mybir

F32 = mybir.dt.float32
BF16 = mybir.dt.bfloat16
FP8 = mybir.dt.float8e4
NP_BF16 = ml_dtypes.bfloat16
NP_FP8 = ml_dtypes.float8_e4m3
W2_SCALE = 256.0

N_CORES = 8
B_FULL = 256
Bs = B_FULL // N_CORES   # 32 batch rows per core
D = 4
H1 = 4096
H2 = 2048
K1 = H1 // 128           # 32 h1 feature chunks
M2 = H2 // 512           # 4 h2 psum tiles
J2 = H2 // 128           # 16 h2 feature chunks
KK = K1 // 2             # 16 Double-FP8 pair chunks


def build_dp(hs):
    T1 = len(hs)
    nc = bacc.Bacc("TRN2", target_bir_lowering=False, debug=False,
                   num_devices=N_CORES)

    d_y0T = nc.dram_tensor("y0T", [D, Bs], F32, kind="ExternalInput").ap()
    d_y0b = nc.dram_tensor("y0b", [5, Bs], BF16, kind="ExternalInput").ap()
    d_w1m = nc.dram_tensor("w1m", [5, H1], BF16, kind="ExternalInput").ap()
    d_w2t = nc.dram_tensor("w2t", [128, K1 * H2], FP8, kind="ExternalInput").ap()
    d_b2p = nc.dram_tensor("b2p", [128, 2 * H2], FP8, kind="ExternalInput").ap()
    d_w3t = nc.dram_tensor("w3t", [128, J2 * D], BF16, kind="ExternalInput").ap()
    d_b3c = nc.dram_tensor("b3c", [D, 1], F32, kind="ExternalInput").ap()
    d_wpa = nc.dram_tensor("wpa", [5, D], BF16, kind="ExternalInput").ap()
    d_wpbs = nc.dram_tensor("wpbs", [D, D], BF16, kind="ExternalInput").ap()
    d_wpbc = nc.dram_tensor("wpbc", [3, D], BF16, kind="ExternalInput").ap()
    d_ones = nc.dram_tensor("ones1", [128, 2 * Bs], FP8, kind="ExternalInput").ap()
    d_i32 = nc.dram_tensor("i32", [Bs, Bs], BF16, kind="ExternalInput").ap()
    d_out = nc.dram_tensor("out", [T1, D, Bs], F32, kind="ExternalOutput").ap()

    with tile.TileContext(nc) as tc:
        with tc.tile_pool(name="wpool", bufs=1) as wp, \
             tc.tile_pool(name="state", bufs=1) as stp, \
             tc.tile_pool(name="act", bufs=2) as actp, \
             tc.tile_pool(name="small", bufs=3) as smp, \
             tc.tile_pool(name="ps_h1", bufs=2, space="PSUM") as ps_h1, \
             tc.tile_pool(name="ps_h2", bufs=2, space="PSUM") as ps_h2, \
             tc.tile_pool(name="ps_tr", bufs=2, space="PSUM") as ps_tr, \
             tc.tile_pool(name="ps_sm", bufs=1, space="PSUM") as ps_sm:

            w1m = wp.tile([5, H1], BF16)
            w2t = wp.tile([128, K1 * H2], FP8)
            b2p = wp.tile([128, 2 * H2], FP8)
            w3t = wp.tile([128, J2 * D], BF16)
            b3c = wp.tile([D, 1], F32)
            wpa = wp.tile([5, D], BF16)
            wpbs = wp.tile([D, D], BF16)
            wpbc = wp.tile([3, D], BF16)
            ones1 = wp.tile([128, 2 * Bs], FP8)
            i32 = wp.tile([Bs, Bs], BF16)
            for t_, d_ in ((w1m, d_w1m), (w2t, d_w2t), (b2p, d_b2p),
                           (w3t, d_w3t), (b3c, d_b3c), (wpa, d_wpa),
                           (wpbs, d_wpbs), (wpbc, d_wpbc), (ones1, d_ones),
                           (i32, d_i32)):
                nc.sync.dma_start(t_[:], d_)

            yb16 = stp.tile([5, Bs], BF16, name="yb16g")
            nc.sync.dma_start(yb16[:], d_y0b)
            ybase = smp.tile([D, Bs], F32, name="ybaseg", tag="ybase")
            nc.sync.dma_start(ybase[:], d_y0T)

            R = mybir.ActivationFunctionType.Relu
            A = mybir.AluOpType
            DR = mybir.MatmulPerfMode.DoubleRow
            ones3 = ones1[:].rearrange("p (j b) -> p j b", j=2)
            bias3 = b2p[:].rearrange("p (j c) -> p j c", j=2)

            def emit_eval(c, ybase, racc, stage):
                """One f-eval + the z-term precompute. Returns poly_ps, th, z."""
                # state-derived small tensors, emitted early (consumed late)
                yshb = actp.tile([3, Bs], BF16, name="yshbg", tag="yshb")
                nc.sync.dma_start(yshb[:], yb16[1:4, :])
                phis = actp.tile([D, Bs], BF16, name="phisg", tag="phis")
                phic = actp.tile([3, Bs], BF16, name="phicg", tag="phic")
                nc.gpsimd.tensor_mul(phis[:], yb16[0:4, :], yb16[0:4, :])
                nc.gpsimd.tensor_mul(phic[:], yb16[0:3, :], yshb[:])

                # h1T chunks: [128, 32] each, 2 psum groups of 16; fp8 out
                # 4 psum tiles of 8 chunks each (own banks) so each relu can
                # fire right after its 8 matmuls; alternate ACT/DVE for the
                # cast so the W2 GEMM starts as soon as chunks 0-7 are ready
                h1b = actp.tile([128, K1 * Bs], FP8, name="h1bg", tag="h1b")
                for g in range(4):
                    h1ps = ps_h1.tile([128, 8 * Bs], F32, name="h1ps", tag="h1ps")
                    for q in range(8):
                        m = g * 8 + q
                        nc.tensor.matmul(h1ps[:, q * Bs:(q + 1) * Bs],
                                         w1m[:, m * 128:(m + 1) * 128],
                                         yb16[:], start=True, stop=True)
                    dst = h1b[:, g * 8 * Bs:(g + 1) * 8 * Bs]
                    if g % 2 == 0:
                        nc.scalar.activation(dst, h1ps[:], R)
                    else:
                        nc.vector.tensor_scalar_max(dst, h1ps[:], 0.0)

                ppT = ps_sm.tile([D, Bs], F32, name="ppg", tag="pp")
                th = smp.tile([D, Bs], F32, name="thg", tag="th")
                poly_ps = z = None
                for m in range(M2):
                    if m == M2 - 1:
                        # poly + z precompute: runs during the last GEMM tile
                        poly_ps = ps_sm.tile([D, Bs], F32, name="polyg", tag="poly")
                        nc.tensor.matmul(poly_ps[:], wpa[:], yb16[:],
                                         start=True, stop=False)
                        nc.tensor.matmul(poly_ps[:], wpbs[:], phis[:],
                                         start=False, stop=False)
                        nc.tensor.matmul(poly_ps[:], wpbc[:], phic[:],
                                         start=False, stop=True)
                        z = smp.tile([D, Bs], F32, name="zg", tag="z")
                        if stage < 3:
                            nc.vector.scalar_tensor_tensor(
                                z[:], poly_ps[:], c, ybase[:],
                                op0=A.mult, op1=A.add)
                        else:
                            zr = smp.tile([D, Bs], F32, name="zrg", tag="zr")
                            nc.vector.scalar_tensor_tensor(
                                zr[:], racc[:], c, ybase[:],
                                op0=A.mult, op1=A.add)
                            nc.vector.scalar_tensor_tensor(
                                z[:], poly_ps[:], c, zr[:],
                                op0=A.mult, op1=A.add)
                    h2ps = ps_h2.tile([Bs, 512], F32, name="h2ps", tag="h2ps")
                    nc.tensor.matmul(h2ps[:], ones3,
                                     bias3[:, :, m * 512:(m + 1) * 512],
                                     start=True, stop=False, perf_mode=DR)
                    for kk in range(KK):
                        lhsT = h1b[:, kk * 2 * Bs:(kk + 1) * 2 * Bs].rearrange(
                            "p (j b) -> p j b", j=2)
                        rhs = w2t[:, kk * 2 * H2:(kk + 1) * 2 * H2].rearrange(
                            "p (j c) -> p j c", j=2)[:, :, m * 512:(m + 1) * 512]
                        nc.tensor.matmul(h2ps[:], lhsT, rhs,
                                         start=False, stop=(kk == KK - 1),
                                         perf_mode=DR)
                    # two halves: relu (ACT | DVE) -> PE transpose -> copy -> W3
                    h2b = actp.tile([Bs, 512], BF16, name="h2bg", tag="h2b")
                    for hf in range(2):
                        dst = h2b[:, hf * 256:(hf + 1) * 256]
                        src = h2ps[:, hf * 256:(hf + 1) * 256]
                        if hf == 0:
                            nc.scalar.activation(dst, src, R)
                        else:
                            nc.vector.tensor_scalar_max(dst, src, 0.0)
                        trps = ps_tr.tile([128, 2 * Bs], BF16,
                                          name="trps", tag="tr")
                        for j2 in range(2):
                            j = hf * 2 + j2
                            nc.tensor.transpose(
                                trps[:, j2 * Bs:(j2 + 1) * Bs],
                                h2b[:, j * 128:(j + 1) * 128], i32[:])
                        h2tb = actp.tile([128, 2 * Bs], BF16,
                                         name="h2tbg", tag="h2tb")
                        if hf == 0:
                            nc.scalar.copy(h2tb[:], trps[:])
                        else:
                            nc.vector.tensor_copy(h2tb[:], trps[:])
                        for j2 in range(2):
                            jj = m * 4 + hf * 2 + j2
                            nc.tensor.matmul(
                                ppT[:], w3t[:, jj * D:(jj + 1) * D],
                                h2tb[:, j2 * Bs:(j2 + 1) * Bs],
                                start=(jj == 0), stop=(jj == J2 - 1))
                nc.scalar.activation(th[:], ppT[:],
                                     mybir.ActivationFunctionType.Tanh,
                                     bias=b3c[:])
                return poly_ps, th, z

            kprev = racc = None
            for t in range(T1):
                h = float(hs[t])
                cs = [h / 2, h / 2, h, h / 6]
                for stage in range(4):
                    c = cs[stage]
                    poly_ps, th, z = emit_eval(c, ybase, racc, stage)
                    # critical path: one fused DVE op to the next state
                    nc.vector.scalar_tensor_tensor(
                        yb16[0:4, :], th[:], c, z[:], op0=A.mult, op1=A.add)
                    if stage == 3:
                        ynew = smp.tile([D, Bs], F32, name="ybaseg", tag="ybase")
                        nc.vector.scalar_tensor_tensor(
                            ynew[:], th[:], c, z[:], op0=A.mult, op1=A.add)
                        ybase = ynew
                        nc.sync.dma_start(d_out[t, :, :], ynew[:])
                    else:
                        # bookkeeping off the critical path
                        k_sb = smp.tile([D, Bs], F32, name="kg", tag="k")
                        nc.vector.tensor_add(k_sb[:], th[:], poly_ps[:])
                        if stage == 0:
                            kprev = k_sb
                        elif stage == 1:
                            r = smp.tile([D, Bs], F32, name="raccg", tag="racc")
                            nc.vector.scalar_tensor_tensor(
                                r[:], k_sb[:], 2.0, kprev[:],
                                op0=A.mult, op1=A.add)
                            racc = r
                        else:
                            r = smp.tile([D, Bs], F32, name="raccg", tag="racc")
                            nc.vector.scalar_tensor_tensor(
                                r[:], k_sb[:], 2.0, racc[:],
                                op0=A.mult, op1=A.add)
                            racc = r
    nc.compile()
    return nc


def prep_inputs(s_grid, y0, W1, b1, W2, b2, W3, b3, wpoly):
    hs = np.diff(np.asarray(s_grid, np.float64)).astype(np.float32)
    y0T = np.asarray(y0, np.float32).T                      # [4, 256]
    w1m = np.concatenate([np.asarray(W1, np.float32).T,
                          np.asarray(b1, np.float32)[None, :]], 0).astype(NP_BF16)
    W2a = np.asarray(W2, np.float32)
    # [p, kk, j, c] pairing layout for Double-FP8: contraction elem (p, j)
    # of pair-chunk kk is h1 dim kk*256 + j*128 + p
    w2tm = np.ascontiguousarray(
        (W2a.T * W2_SCALE).reshape(K1 // 2, 2, 128, H2)
        .transpose(2, 0, 1, 3).reshape(128, K1 * H2)
    ).astype(NP_FP8)
    # bias pair-chunk: contributes b2*256 once (row p=0, j=0), zero elsewhere
    b2pm = np.zeros((128, 2, H2), np.float32)
    b2pm[0, 0, :] = np.asarray(b2, np.float32) * W2_SCALE
    b2pm = b2pm.reshape(128, 2 * H2).astype(NP_FP8)
    W3a = np.asarray(W3, np.float32)
    w3tm = np.ascontiguousarray(
        (W3a.T / W2_SCALE).reshape(J2, 128, D).transpose(1, 0, 2).reshape(128, J2 * D)
    ).astype(NP_BF16)
    b3c = np.asarray(b3, np.float32)[:, None]
    w = np.asarray(wpoly, np.float32)
    wpa = np.zeros((5, 4), np.float32)
    wpb = np.zeros((7, 4), np.float32)
    wpa[4, 0] = w[0]; wpa[0, 0] = w[1]; wpb[0, 0] = w[2]
    wpa[4, 1] = w[3]; wpa[0, 1] = w[4]; wpb[0, 1] = w[5]
    wpa[1, 1] = w[6]; wpb[1, 1] = w[7]; wpb[4, 1] = w[8]
    wpa[4, 2] = w[9]; wpa[2, 2] = w[10]; wpb[2, 2] = w[11]
    wpa[1, 2] = w[12]; wpb[1, 2] = w[13]; wpb[5, 2] = w[14]
    wpa[4, 3] = w[15]; wpa[3, 3] = w[16]; wpb[3, 3] = w[17]
    wpa[2, 3] = w[18]; wpb[2, 3] = w[19]; wpb[6, 3] = w[20]
    wpbs = wpb[0:4].astype(NP_BF16)
    wpbc = wpb[4:7].astype(NP_BF16)
    wpa = wpa.astype(NP_BF16)
    ones1 = np.ones((128, 2 * Bs), np.float32).astype(NP_FP8)
    i32 = np.eye(Bs, dtype=np.float32).astype(NP_BF16)
    in_maps = []
    for c in range(N_CORES):
        y0T_c = np.ascontiguousarray(y0T[:, c * Bs:(c + 1) * Bs])
        y0b5 = np.concatenate([y0T_c, np.ones((1, Bs), np.float32)],
                              0).astype(NP_BF16)
        in_maps.append({
            "y0T": y0T_c, "y0b": y0b5, "w1m": w1m, "w2t": w2tm, "b2p": b2pm,
            "w3t": w3tm, "b3c": b3c, "wpa": wpa, "wpbs": wpbs, "wpbc": wpbc,
            "ones1": ones1, "i32": i32,
        })
    return hs, in_maps


def assemble(results, y0):
    ys = np.stack([results[c]["out"] for c in range(N_CORES)])  # [8, T1, 4, 32]
    ys = ys.transpose(1, 0, 3, 2).reshape(ys.shape[1], B_FULL, D)
    return np.concatenate([np.asarray(y0, np.float32)[None], ys], 0)


_CACHE = {}


def kernel(s_grid, y0, W1, b1, W2, b2, W3, b3, wpoly):
    """Full-input, full-output entry point. Returns [T, 256, 4] float32."""
    import os
    os.environ.setdefault("NEURON_RT_RESET_CORES", "1")
    hs, in_maps = prep_inputs(s_grid, y0, W1, b1, W2, b2, W3, b3, wpoly)
    key = tuple(np.asarray(hs, np.float64).round(12).tolist())
    if key not in _CACHE:
        _CACHE[key] = build_dp(hs)
    nc = _CACHE[key]
    from concourse import bass_utils
    res = None
    for attempt in range(3):
        try:
            res = bass_utils.run_bass_kernel_spmd(
                nc, in_maps, core_ids=list(range(N_CORES)))
            break
        except Exception:
            if attempt == 2:
                raise
    results = {c: res.results[c] for c in range(N_CORES)}
    return assemble(results, y0).astype(np.float32)


# revision 21
# speedup vs baseline: 2.5723x; 2.5723x over previous
"""TRN2 Bass kernel for nn_ODEModel (RK4 neural ODE, dense MLP vector field).

Strategy: 8-way DATA-parallel over the batch (32 rows/core), zero collectives.
All weights SBUF-resident (W2^T in fp8 = 8MB).  Per f-eval on each core:
  h1T = relu(W1^T chunks @ y)          32x [128,32] psum tiles, feature-major,
                                       cast to fp8 (e4m3)
  h2' = relu(h1 @ (256*W2^T) + 256*b2) batch-major [32, 2048] via Double-FP8
                                       (256-dim contraction per mm); bias as a
                                       17th DoubleRow mm; scale folded into W3
  h2T = PE-transpose(h2')              16x [32,128]->[128,32]
  pp  = (W3^T/256) chunks @ h2T        accumulated [4,32]; tanh(pp+b3) on ACT
  k   = tanh + poly(y)                 poly via 3 tiny matmuls, emitted late
RK4 combination in fp32 (master state); z-terms precomputed during the last
GEMM tile so only one DVE op separates tanh from the next eval's matmuls.
Output stored as [T-1, 4, 32] per core; host transposes/concats and
prepends y0.
"""
import sys

sys.path.insert(0, "/opt/trn_rl_repo")
import numpy as np
import ml_dtypes

import concourse.bass as bass
import concourse.bacc as bacc
import concourse.tile as tile
import concourse.mybir as mybir

F32 = mybir.dt.float32
BF16 = mybir.dt.bfloat16
FP8 = mybir.dt.float8e4
NP_BF16 = ml_dtypes.bfloat16
NP_FP8 = ml_dtypes.float8_e4m3
W2_SCALE = 256.0

N_CORES = 8
B_FULL = 256
Bs = B_FULL // N_CORES   # 32 batch rows per core
D = 4
H1 = 4096
H2 = 2048
K1 = H1 // 128           # 32 h1 feature chunks
M2 = H2 // 512           # 4 h2 psum tiles
J2 = H2 // 128           # 16 h2 feature chunks
KK = K1 // 2             # 16 Double-FP8 pair chunks


def build_dp(hs):
    T1 = len(hs)
    nc = bacc.Bacc("TRN2", target_bir_lowering=False, debug=False,
                   num_devices=N_CORES)

    d_y0T = nc.dram_tensor("y0T", [D, Bs], F32, kind="ExternalInput").ap()
    d_y0b = nc.dram_tensor("y0b", [5, Bs], BF16, kind="ExternalInput").ap()
    d_w1m = nc.dram_tensor("w1m", [5, H1], BF16, kind="ExternalInput").ap()
    d_w2t = nc.dram_tensor("w2t", [128, (K1 + 2) * H2], FP8,
                           kind="ExternalInput").ap()
    d_w3t = nc.dram_tensor("w3t", [128, J2 * D], BF16, kind="ExternalInput").ap()
    d_b3c = nc.dram_tensor("b3c", [D, 1], F32, kind="ExternalInput").ap()
    d_wpa = nc.dram_tensor("wpa", [5, D], BF16, kind="ExternalInput").ap()
    d_wpbs = nc.dram_tensor("wpbs", [D, D], BF16, kind="ExternalInput").ap()
    d_wpbc = nc.dram_tensor("wpbc", [3, D], BF16, kind="ExternalInput").ap()
    d_ones = nc.dram_tensor("ones1", [128, 2 * Bs], FP8, kind="ExternalInput").ap()
    d_i32 = nc.dram_tensor("i32", [Bs, Bs], BF16, kind="ExternalInput").ap()
    d_out = nc.dram_tensor("out", [T1, D, Bs], F32, kind="ExternalOutput").ap()

    with tile.TileContext(nc) as tc:
        with tc.tile_pool(name="wpool", bufs=1) as wp, \
             tc.tile_pool(name="state", bufs=1) as stp, \
             tc.tile_pool(name="act", bufs=2) as actp, \
             tc.tile_pool(name="small", bufs=3) as smp, \
             tc.tile_pool(name="ps_h1", bufs=2, space="PSUM") as ps_h1, \
             tc.tile_pool(name="ps_h2", bufs=2, space="PSUM") as ps_h2, \
             tc.tile_pool(name="ps_tr", bufs=2, space="PSUM") as ps_tr, \
             tc.tile_pool(name="ps_sm", bufs=1, space="PSUM") as ps_sm:

            w1m = wp.tile([5, H1], BF16)
            w2t = wp.tile([128, (K1 + 2) * H2], FP8)
            w3t = wp.tile([128, J2 * D], BF16)
            b3c = wp.tile([D, 1], F32)
            wpa = wp.tile([5, D], BF16)
            wpbs = wp.tile([D, D], BF16)
            wpbc = wp.tile([3, D], BF16)
            i32 = wp.tile([Bs, Bs], BF16)
            for t_, d_ in ((w1m, d_w1m), (w2t, d_w2t),
                           (w3t, d_w3t), (b3c, d_b3c), (wpa, d_wpa),
                           (wpbs, d_wpbs), (wpbc, d_wpbc),
                           (i32, d_i32)):
                nc.sync.dma_start(t_[:], d_)

            # h1 activations in fp8, single fixed buffer; chunks K1..K1+1 are
            # constant ones feeding the bias pair-chunk of w2t
            h1b = wp.tile([128, (K1 + 2) * Bs], FP8)
            nc.sync.dma_start(h1b[:, K1 * Bs:(K1 + 2) * Bs], d_ones)

            yb16 = stp.tile([5, Bs], BF16, name="yb16g")
            nc.sync.dma_start(yb16[:], d_y0b)
            ybase = smp.tile([D, Bs], F32, name="ybaseg", tag="ybase")
            nc.sync.dma_start(ybase[:], d_y0T)

            R = mybir.ActivationFunctionType.Relu
            A = mybir.AluOpType
            DR = mybir.MatmulPerfMode.DoubleRow

            def emit_eval(c, ybase, racc, stage):
                """One f-eval + the z-term precompute. Returns poly_ps, th, z."""
                # state-derived small tensors, emitted early (consumed late)
                yshb = actp.tile([3, Bs], BF16, name="yshbg", tag="yshb")
                nc.sync.dma_start(yshb[:], yb16[1:4, :])
                phis = actp.tile([D, Bs], BF16, name="phisg", tag="phis")
                phic = actp.tile([3, Bs], BF16, name="phicg", tag="phic")
                nc.gpsimd.tensor_mul(phis[:], yb16[0:4, :], yb16[0:4, :])
                nc.gpsimd.tensor_mul(phic[:], yb16[0:3, :], yshb[:])

                # h1T chunks: [128, 32] each, 2 psum groups of 16; fp8 out
                # 4 psum tiles of 8 chunks each (own banks) so each relu can
                # fire right after its 8 matmuls; alternate ACT/DVE for the
                # cast so the W2 GEMM starts as soon as chunks 0-7 are ready
                for g in range(4):
                    h1ps = ps_h1.tile([128, 8 * Bs], F32, name="h1ps", tag="h1ps")
                    for q in range(8):
                        m = g * 8 + q
                        nc.tensor.matmul(h1ps[:, q * Bs:(q + 1) * Bs],
                                         w1m[:, m * 128:(m + 1) * 128],
                                         yb16[:], start=True, stop=True)
                    dst = h1b[:, g * 8 * Bs:(g + 1) * 8 * Bs]
                    if g % 2 == 0:
                        nc.scalar.activation(dst, h1ps[:], R)
                    else:
                        nc.vector.tensor_scalar_max(dst, h1ps[:], 0.0)

                ppT = ps_sm.tile([D, Bs], F32, name="ppg", tag="pp")
                th = smp.tile([D, Bs], F32, name="thg", tag="th")
                poly_ps = z = None
                for m in range(M2):
                    if m == M2 - 1:
                        # poly + z precompute: runs during the last GEMM tile
                        poly_ps = ps_sm.tile([D, Bs], F32, name="polyg", tag="poly")
                        nc.tensor.matmul(poly_ps[:], wpa[:], yb16[:],
                                         start=True, stop=False)
                        nc.tensor.matmul(poly_ps[:], wpbs[:], phis[:],
                                         start=False, stop=False)
                        nc.tensor.matmul(poly_ps[:], wpbc[:], phic[:],
                                         start=False, stop=True)
                        z = smp.tile([D, Bs], F32, name="zg", tag="z")
                        if stage < 3:
                            nc.vector.scalar_tensor_tensor(
                                z[:], poly_ps[:], c, ybase[:],
                                op0=A.mult, op1=A.add)
                        else:
                            zr = smp.tile([D, Bs], F32, name="zrg", tag="zr")
                            nc.vector.scalar_tensor_tensor(
                                zr[:], racc[:], c, ybase[:],
                                op0=A.mult, op1=A.add)
                            nc.vector.scalar_tensor_tensor(
                                z[:], poly_ps[:], c, zr[:],
                                op0=A.mult, op1=A.add)
                    h2ps = ps_h2.tile([Bs, 512], F32, name="h2ps", tag="h2ps")
                    for kk in range(KK + 1):
                        lhsT = h1b[:, kk * 2 * Bs:(kk + 1) * 2 * Bs].rearrange(
                            "p (j b) -> p j b", j=2)
                        rhs = w2t[:, kk * 2 * H2:(kk + 1) * 2 * H2].rearrange(
                            "p (j c) -> p j c", j=2)[:, :, m * 512:(m + 1) * 512]
                        nc.tensor.matmul(h2ps[:], lhsT, rhs,
                                         start=(kk == 0), stop=(kk == KK),
                                         perf_mode=DR)
                    # two halves: relu (ACT | DVE) -> PE transpose -> copy -> W3
                    h2b = actp.tile([Bs, 512], BF16, name="h2bg", tag="h2b")
                    for hf in range(2):
                        dst = h2b[:, hf * 256:(hf + 1) * 256]
                        src = h2ps[:, hf * 256:(hf + 1) * 256]
                        if hf == 0:
                            nc.scalar.activation(dst, src, R)
                        else:
                            nc.vector.tensor_scalar_max(dst, src, 0.0)
                        trps = ps_tr.tile([128, 2 * Bs], BF16,
                                          name="trps", tag="tr")
                        for j2 in range(2):
                            j = hf * 2 + j2
                            nc.tensor.transpose(
                                trps[:, j2 * Bs:(j2 + 1) * Bs],
                                h2b[:, j * 128:(j + 1) * 128], i32[:])
                        h2tb = actp.tile([128, 2 * Bs], BF16,
                                         name="h2tbg", tag="h2tb")
                        if hf == 0:
                            nc.scalar.copy(h2tb[:], trps[:])
                        else:
                            nc.vector.tensor_copy(h2tb[:], trps[:])
                        for j2 in range(2):
                            jj = m * 4 + hf * 2 + j2
                            nc.tensor.matmul(
                                ppT[:], w3t[:, jj * D:(jj + 1) * D],
                                h2tb[:, j2 * Bs:(j2 + 1) * Bs],
                                start=(jj == 0), stop=(jj == J2 - 1))
                nc.scalar.activation(th[:], ppT[:],
                                     mybir.ActivationFunctionType.Tanh,
                                     bias=b3c[:])
                return poly_ps, th, z

            kprev = racc = None
            for t in range(T1):
                h = float(hs[t])
                cs = [h / 2, h / 2, h, h / 6]
                for stage in range(4):
                    c = cs[stage]
                    poly_ps, th, z = emit_eval(c, ybase, racc, stage)
                    # critical path: one fused DVE op to the next state
                    nc.vector.scalar_tensor_tensor(
                        yb16[0:4, :], th[:], c, z[:], op0=A.mult, op1=A.add)
                    if stage == 3:
                        ynew = smp.tile([D, Bs], F32, name="ybaseg", tag="ybase")
                        nc.vector.scalar_tensor_tensor(
                            ynew[:], th[:], c, z[:], op0=A.mult, op1=A.add)
                        ybase = ynew
                        nc.sync.dma_start(d_out[t, :, :], ynew[:])
                    else:
                        # bookkeeping off the critical path
                        k_sb = smp.tile([D, Bs], F32, name="kg", tag="k")
                        nc.vector.tensor_add(k_sb[:], th[:], poly_ps[:])
                        if stage == 0:
                            kprev = k_sb
                        elif stage == 1:
                            r = smp.tile([D, Bs], F32, name="raccg", tag="racc")
                            nc.vector.scalar_tensor_tensor(
                                r[:], k_sb[:], 2.0, kprev[:],
                                op0=A.mult, op1=A.add)
                            racc = r
                        else:
                            r = smp.tile([D, Bs], F32, name="raccg", tag="racc")
                            nc.vector.scalar_tensor_tensor(
                                r[:], k_sb[:], 2.0, racc[:],
                                op0=A.mult, op1=A.add)
                            racc = r
    nc.compile()
    return nc


def prep_inputs(s_grid, y0, W1, b1, W2, b2, W3, b3, wpoly):
    hs = np.diff(np.asarray(s_grid, np.float64)).astype(np.float32)
    y0T = np.asarray(y0, np.float32).T                      # [4, 256]
    w1m = np.concatenate([np.asarray(W1, np.float32).T,
                          np.asarray(b1, np.float32)[None, :]], 0).astype(NP_BF16)
    W2a = np.asarray(W2, np.float32)
    # [p, kk, j, c] pairing layout for Double-FP8: contraction elem (p, j)
    # of pair-chunk kk is h1 dim kk*256 + j*128 + p.  Pair-chunk K1//2 is the
    # bias: b2*256 at (p=0, j=0), zero elsewhere (h1b holds ones there).
    w2tm = np.zeros((128, K1 // 2 + 1, 2, H2), np.float32)
    w2tm[:, :K1 // 2] = (W2a.T * W2_SCALE).reshape(K1 // 2, 2, 128, H2)\
        .transpose(2, 0, 1, 3)
    w2tm[0, K1 // 2, 0, :] = np.asarray(b2, np.float32) * W2_SCALE
    w2tm = np.ascontiguousarray(w2tm.reshape(128, (K1 + 2) * H2)).astype(NP_FP8)
    W3a = np.asarray(W3, np.float32)
    w3tm = np.ascontiguousarray(
        (W3a.T / W2_SCALE).reshape(J2, 128, D).transpose(1, 0, 2).reshape(128, J2 * D)
    ).astype(NP_BF16)
    b3c = np.asarray(b3, np.float32)[:, None]
    w = np.asarray(wpoly, np.float32)
    wpa = np.zeros((5, 4), np.float32)
    wpb = np.zeros((7, 4), np.float32)
    wpa[4, 0] = w[0]; wpa[0, 0] = w[1]; wpb[0, 0] = w[2]
    wpa[4, 1] = w[3]; wpa[0, 1] = w[4]; wpb[0, 1] = w[5]
    wpa[1, 1] = w[6]; wpb[1, 1] = w[7]; wpb[4, 1] = w[8]
    wpa[4, 2] = w[9]; wpa[2, 2] = w[10]; wpb[2, 2] = w[11]
    wpa[1, 2] = w[12]; wpb[1, 2] = w[13]; wpb[5, 2] = w[14]
    wpa[4, 3] = w[15]; wpa[3, 3] = w[16]; wpb[3, 3] = w[17]
    wpa[2, 3] = w[18]; wpb[2, 3] = w[19]; wpb[6, 3] = w[20]
    wpbs = wpb[0:4].astype(NP_BF16)
    wpbc = wpb[4:7].astype(NP_BF16)
    wpa = wpa.astype(NP_BF16)
    ones1 = np.ones((128, 2 * Bs), np.float32).astype(NP_FP8)
    i32 = np.eye(Bs, dtype=np.float32).astype(NP_BF16)
    in_maps = []
    for c in range(N_CORES):
        y0T_c = np.ascontiguousarray(y0T[:, c * Bs:(c + 1) * Bs])
        y0b5 = np.concatenate([y0T_c, np.ones((1, Bs), np.float32)],
                              0).astype(NP_BF16)
        in_maps.append({
            "y0T": y0T_c, "y0b": y0b5, "w1m": w1m, "w2t": w2tm,
            "w3t": w3tm, "b3c": b3c, "wpa": wpa, "wpbs": wpbs, "wpbc": wpbc,
            "ones1": ones1, "i32": i32,
        })
    return hs, in_maps


def assemble(results, y0):
    ys = np.stack([results[c]["out"] for c in range(N_CORES)])  # [8, T1, 4, 32]
    ys = ys.transpose(1, 0, 3, 2).reshape(ys.shape[1], B_FULL, D)
    return np.concatenate([np.asarray(y0, np.float32)[None], ys], 0)


_CACHE = {}


def kernel(s_grid, y0, W1, b1, W2, b2, W3, b3, wpoly):
    """Full-input, full-output entry point. Returns [T, 256, 4] float32."""
    import os
    os.environ.setdefault("NEURON_RT_RESET_CORES", "1")
    hs, in_maps = prep_inputs(s_grid, y0, W1, b1, W2, b2, W3, b3, wpoly)
    key = tuple(np.asarray(hs, np.float64).round(12).tolist())
    if key not in _CACHE:
        _CACHE[key] = build_dp(hs)
    nc = _CACHE[key]
    from concourse import bass_utils
    res = None
    for attempt in range(3):
        try:
            res = bass_utils.run_bass_kernel_spmd(
                nc, in_maps, core_ids=list(range(N_CORES)))
            break
        except Exception:
            if attempt == 2:
                raise
    results = {c: res.results[c] for c in range(N_CORES)}
    return assemble(results, y0).astype(np.float32)
